# revision 41
# baseline (speedup 1.0000x reference)
"""Trainium2 Bass kernel for nn_MultiHeadAttention_86079734546451.

Sharding: data-parallel over batch B=16 across 8 cores (2 batches/core).
All weights replicated. No collectives.

Per-core math (B_loc=2, D=512, N=128 nodes, S=14, L=12, H=8, dh=64):
  qh/kh = d-major [dout, (n,s)] bf16 projections (scale folded into Wq).
  bias  = einsum('lnmh,sl->mns', ab, Wd) via per-(h,nb) kron matmuls:
     lhsT = host-pretransposed ab block [(l,ni)=96, m=128] bf16,
     rhs  = kron Wd [96, (ni,s16)];  4 blocks -> one [128,512] psum tile,
     ACT-copied (strided, dropping the s-pad) into the score psum as the
     accumulation INIT; score matmuls then accumulate with start=False.
     (bd cancels in the softmax and is dropped.)
  softmax over the query axis == free-axis softmax in the [m,(s,n)] layout:
     one exp [128,1792] -> ebt bf16, 3D reduce -> Z, reciprocal,
     gpsimd broadcast-multiply onto vh -> vp; AV matmuls per (h,s);
     O-projection from the d-major yt tiles.
"""

import sys

sys.path.insert(0, "/opt/trn_rl_repo")

from contextlib import ExitStack

import numpy as np
import ml_dtypes

import concourse.bass as bass
import concourse.mybir as mybir
import concourse.tile as tile
from concourse import bacc
from concourse.bass import broadcast_tensor_aps

f32 = mybir.dt.float32
bf16 = mybir.dt.bfloat16
AF = mybir.ActivationFunctionType

# Problem constants
B_LOC = 2          # batches per core
D = 512
N = 128            # nodes
S = 14             # seq
SP = 14            # kron weight s-extent (no pad; 4x448B fits one bank)
L = 12
H = 8
DH = 64            # head dim
TOK = N * S        # 1792 tokens per batch, (n, s) order
C = 4              # 128-chunks of D
NCORES = 8
NI = 8             # n per kron block
NBK = N // NI      # 16 kron blocks
K96 = L * NI       # kron contraction size (real rows)
KB = 128           # padded kron K (zero rows 96..127) -> enables PE FWL


def emit_kernel(ctx: ExitStack, tc: "tile.TileContext", io: dict):
    nc = tc.nc

    q_d, k_d, v_d, ab_d = io["q"], io["k"], io["v"], io["abk"]
    out_d = io["out"]

    # ---------------- pools ----------------
    wpool = ctx.enter_context(tc.tile_pool(name="wpool", bufs=1))
    xin = ctx.enter_context(tc.tile_pool(name="xin", bufs=14))
    qkh = ctx.enter_context(tc.tile_pool(name="qkh", bufs=16))
    vhp = ctx.enter_context(tc.tile_pool(name="vhp", bufs=1))
    abp = ctx.enter_context(tc.tile_pool(name="abp", bufs=3))
    ebp = ctx.enter_context(tc.tile_pool(name="ebp", bufs=3))
    vpp = ctx.enter_context(tc.tile_pool(name="vpp", bufs=3))
    zrp = ctx.enter_context(tc.tile_pool(name="zrp", bufs=4))
    ytp = ctx.enter_context(tc.tile_pool(name="ytp", bufs=4))
    osbp = ctx.enter_context(tc.tile_pool(name="osbp", bufs=3))
    bhp = ctx.enter_context(tc.tile_pool(name="bhp", bufs=6))

    pp = ctx.enter_context(tc.tile_pool(name="pp", bufs=2, space="PSUM"))
    scp = ctx.enter_context(tc.tile_pool(name="scp", bufs=2, space="PSUM"))
    avp = ctx.enter_context(tc.tile_pool(name="avp", bufs=2, space="PSUM"))

    # ---------------- weights (once) ----------------
    wq, wk, wv, wo = [], [], [], []
    for nm, lst in (("wqT", wq), ("wkT", wk), ("wvT", wv), ("woT", wo)):
        for c in range(C):
            w_c = wpool.tile([128, D], bf16, name=f"{nm}{c}", tag=f"{nm}{c}")
            nc.scalar.dma_start(w_c[:], io[nm][c * 128:(c + 1) * 128, :])
            lst.append(w_c)

    wdh = wpool.tile([KB, NI * SP], bf16, name="wdh", tag="wdh")
    nc.scalar.dma_start(wdh[:], io["wdh"][:])

    # packed per-partition consts: cols 0-3 bq chunks, 4-7 bk chunks
    cst = wpool.tile([128, 8], f32, name="cst", tag="cst")
    nc.scalar.dma_start(cst[:], io["cst"][:])

    # broadcast bv/bo along partitions via K=1 ones-matmul
    ones = wpool.tile([1, 128], f32, name="ones", tag="ones")
    nc.vector.memset(ones[:], 1.0)
    bv_st = wpool.tile([1, D], f32, name="bv_st", tag="bv_st")
    nc.scalar.dma_start(bv_st[:], io["bv"].unsqueeze(0))
    bo_st = wpool.tile([1, D], f32, name="bo_st", tag="bo_st")
    nc.scalar.dma_start(bo_st[:], io["bo"].unsqueeze(0))
    bvb = wpool.tile([128, D], f32, name="bvb", tag="bvb")
    bob = wpool.tile([128, D], f32, name="bob", tag="bob")
    for src, dst in ((bv_st, bvb), (bo_st, bob)):
        pst = pp.tile([128, D], f32, tag="pp", name="p_bcast")
        nc.tensor.matmul(pst[:], lhsT=ones[:], rhs=src[:], start=True, stop=True)
        nc.vector.tensor_copy(dst[:], pst[:])

    def load_x(b, split=False):
        # x tiles are [128, (s, n)] -- host pre-transposed to [B, D, S, N]
        # split=True loads each chunk in column halves so the first proj
        # matmuls can start before the whole tile has landed (cold start)
        xs = {}
        for (src_d, nm) in ((q_d, "xq"), (k_d, "xk"), (v_d, "xv")):
            lst = []
            for ci in range(C):
                x_c = xin.tile([128, TOK], bf16, tag="xin", name=f"{nm}{ci}")
                src = src_d[b, ci * 128:(ci + 1) * 128].rearrange(
                    "p s n -> p (s n)")
                if split and nm != "xv":
                    nc.sync.dma_start(x_c[:, :448], src[:, :448])
                    nc.sync.dma_start(x_c[:, 448:896], src[:, 448:896])
                    nc.sync.dma_start(x_c[:, 896:], src[:, 896:])
                else:
                    nc.sync.dma_start(x_c[:], src)
                lst.append(x_c)
            xs[nm] = lst
        return xs

    def load_ab(b, h):
        ab_h = abp.tile([KB, NBK * 128], bf16, tag="abt", name=f"ab{h}")
        nc.sync.dma_start(ab_h[:], ab_d[b, h])
        return ab_h

    def emit_p1qk(xq, xk):
        # Q and K projections -> d-major [dout, (s,n)] bf16.
        # 4 concurrent psum accumulation chains (2 from pp + 2 from the
        # P3-idle avp pool) so the PE streams without RAW stalls
        qh, kh = [], []
        for pi, (xin_l, wts, dst_list) in enumerate(
                ((xq, wq, qh), (xk, wk, kh))):
            for co in range(C):
                h_c = qkh.tile([128, TOK], bf16, tag="qkh", name=f"h{co}")
                pss = [(pp if tb < 2 else avp).tile(
                    [128, 448], f32, tag="pp" if tb < 2 else "av",
                    name="ps_qk") for tb in range(4)]
                for ci in range(C):
                    for tb in range(4):
                        nc.tensor.matmul(
                            pss[tb][:],
                            lhsT=wts[ci][:, co * 128:(co + 1) * 128],
                            rhs=xin_l[ci][:, tb * 448:(tb + 1) * 448],
                            start=(ci == 0), stop=(ci == C - 1))
                for tb in range(4):
                    if pi == 0:
                        nc.scalar.activation(
                            h_c[:, tb * 448:(tb + 1) * 448], pss[tb][:],
                            AF.Identity, bias=cst[:, co:co + 1], scale=1.0)
                    else:
                        nc.vector.tensor_scalar_add(
                            h_c[:, tb * 448:(tb + 1) * 448], pss[tb][:],
                            cst[:, 4 + co:4 + co + 1])
                dst_list.append(h_c)
        return qh, kh

    # ---------------- per-batch body ----------------
    xs = load_x(0, split=True)
    qk_pre = None
    for b in range(B_LOC):
        xq, xk, xv = xs["xq"], xs["xk"], xs["xv"]

        # ---- P1: Q/K (emitted early during the previous batch's tail)
        qh, kh = qk_pre if qk_pre is not None else emit_p1qk(xq, xk)

        # ---- P1b: V projection -> token-major vh [n, (s,d)] bf16 (+bv)
        vh = vhp.tile([128, S * D], bf16, tag="vh", name="vh")
        for s0 in range(0, S, 4):
            sw = min(4, S - s0)
            pss = [(pp if sj < 2 else avp).tile(
                [128, D], f32, tag="pp" if sj < 2 else "av",
                name="ps_v") for sj in range(sw)]
            for ci in range(C):
                for sj in range(sw):
                    s = s0 + sj
                    nc.tensor.matmul(
                        pss[sj][:],
                        lhsT=xv[ci][:, s * 128:(s + 1) * 128],
                        rhs=wv[ci][:],
                        start=(ci == 0), stop=(ci == C - 1))
            for sj in range(sw):
                s = s0 + sj
                nc.vector.tensor_add(vh[:, s * D:(s + 1) * D], pss[sj][:],
                                     bvb[:])

        # first two ab loads ahead of the bulky next-batch x prefetch
        abL = {hh: load_ab(b, hh) for hh in range(2)}
        # prefetch next batch's inputs (sync queue, before this batch's outs)
        if b + 1 < B_LOC:
            xs = load_x(b + 1)

        # ---- P3: attention per head
        yt = [ytp.tile([128, TOK], bf16, tag="ytp", name=f"yt{c}")
              for c in range(C)]
        ebts, vps = {}, {}

        def av_block(pair):
            # heads of a pair write distinct out-partition halves (col_grp
            # 0 vs 64) -- interleave them pairwise so the PE co-runs them
            cc = pair
            for g in range(4):
                s0 = g * 4
                sw = min(4, S - s0)
                av = avp.tile([128, 512], f32, tag="av", name="av")
                for si in range(sw):
                    s = s0 + si
                    for hh in (2 * cc, 2 * cc + 1):
                        hbb = (hh % 2) * DH
                        nc.tensor.matmul(
                            av[hbb:hbb + DH, si * 128:(si + 1) * 128],
                            lhsT=vps[hh][:, s * DH:(s + 1) * DH],
                            rhs=ebts[hh][:, s * 128:(s + 1) * 128],
                            start=True, stop=True, skip_group_check=True)
                nc.scalar.copy(
                    yt[cc][:, s0 * 128:(s0 + sw) * 128],
                    av[:, :sw * 128])

        for h in range(H):
            c = h // 2
            hb = (h % 2) * DH
            if h + 2 < H:
                abL[h + 2] = load_ab(b, h + 2)
            # bias kron matmuls -> [128,512] psum tiles -> sbuf bf16
            # (engine-write -> PE-accumulate proved racy on HW, so the
            # bias is added with DVE after the score matmuls instead)
            bias_sb = []
            for g in range(4):
                psb = pp.tile([128, 4 * NI * SP], f32, tag="pp", name="ps_b")
                for j in range(4):
                    nb = g * 4 + j
                    nc.tensor.matmul(
                        psb[:, j * NI * SP:(j + 1) * NI * SP],
                        lhsT=abL[h][:, nb * 128:(nb + 1) * 128],
                        rhs=wdh[:],
                        start=True, stop=True, skip_group_check=True)
                bs = bhp.tile([128, 4 * NI * SP], bf16, tag="bh", name="bias_sb")
                nc.scalar.copy(bs[:], psb[:])
                bias_sb.append(bs)
            # AV of the previous pair slots in here: its vp/ebt chain has
            # had a full head iteration to finish, so PE never stalls on it
            if h % 2 == 0 and h >= 2:
                av_block(h // 2 - 1)
            # scores + softmax, split into s-halves A (s 0..6) and B
            # (s 7..13) so the next head's A matmuls only wait on exp(A)
            ebt = ebp.tile([128, TOK], bf16, tag="eb", name="ebt")
            zt = zrp.tile([128, S], f32, tag="z", name="zt")
            rt = zrp.tile([128, S], f32, tag="r", name="rt")
            vp = vpp.tile([128, S * DH], bf16, tag="vp", name="vp")
            HS = S // 2  # 7
            schs = [scp.tile([128, HS * 128], f32, tag="sc", name="sc")
                    for _ in range(2)]
            for half in range(2):
                sch = schs[half]
                sb = half * HS
                for si in range(HS):
                    s = sb + si
                    nc.tensor.matmul(
                        sch[:, si * 128:(si + 1) * 128],
                        lhsT=kh[c][hb:hb + DH, s * 128:(s + 1) * 128],
                        rhs=qh[c][hb:hb + DH, s * 128:(s + 1) * 128],
                        start=True, stop=True, skip_group_check=True)
            for half in range(2):
                sch = schs[half]
                sb = half * HS
                sc_ns = sch.rearrange("p (s n) -> p n s", s=HS)
                for g in range(4):
                    nc.vector.tensor_add(
                        sc_ns[:, g * 32:(g + 1) * 32, :],
                        sc_ns[:, g * 32:(g + 1) * 32, :],
                        bias_sb[g].rearrange("p (n s) -> p n s", s=SP)[
                            :, :, sb:sb + HS])
                nc.scalar.activation(ebt[:, sb * 128:(sb + HS) * 128],
                                     sch[:], AF.Exp)
                nc.vector.reduce_sum(
                    zt[:, sb:sb + HS].unsqueeze(2),
                    ebt.rearrange("p (s n) -> p s n", s=S)[:, sb:sb + HS],
                    axis=mybir.AxisListType.X)
                nc.vector.reciprocal(rt[:, sb:sb + HS], zt[:, sb:sb + HS])
                vsrc = vh.rearrange("p (s d) -> p s d", s=S)[
                    :, sb:sb + HS, h * DH:(h + 1) * DH]
                a2, b2 = broadcast_tensor_aps(
                    vsrc, rt[:, sb:sb + HS].unsqueeze(2))
                nc.gpsimd.tensor_mul(
                    vp.rearrange("p (s d) -> p s d", s=S)[:, sb:sb + HS],
                    a2, b2)
            ebts[h], vps[h] = ebt, vp
        # fill the last pair's chain latency with next batch's QK proj
        qk_pre = emit_p1qk(xs["xq"], xs["xk"]) if b + 1 < B_LOC else None
        av_block(3)

        # ---- P4: output projection (4 concurrent psum chains)
        for s0 in range(0, S, 4):
            sw = min(4, S - s0)
            pss = [(pp if sj < 2 else avp).tile(
                [128, D], f32, tag="pp" if sj < 2 else "av",
                name="ps_o") for sj in range(sw)]
            for ci in range(C):
                for sj in range(sw):
                    s = s0 + sj
                    nc.tensor.matmul(
                        pss[sj][:],
                        lhsT=yt[ci][:, s * 128:(s + 1) * 128],
                        rhs=wo[ci][:],
                        start=(ci == 0), stop=(ci == C - 1))
            for sj in range(sw):
                s = s0 + sj
                osb = osbp.tile([128, D], f32, tag="osb", name="osb")
                nc.vector.tensor_add(osb[:], pss[sj][:], bob[:])
                nc.sync.dma_start(out_d[b, s], osb[:])


def build_nc():
    nc = bacc.Bacc("TRN2", target_bir_lowering=False, debug=False,
                   num_devices=NCORES)
    io = {}
    io["q"] = nc.dram_tensor("q", [B_LOC, D, S, N], bf16, kind="ExternalInput").ap()
    io["k"] = nc.dram_tensor("k", [B_LOC, D, S, N], bf16, kind="ExternalInput").ap()
    io["v"] = nc.dram_tensor("v", [B_LOC, D, S, N], bf16, kind="ExternalInput").ap()
    io["abk"] = nc.dram_tensor("abk", [B_LOC, H, KB, NBK * 128], bf16,
                               kind="ExternalInput").ap()
    for nm in ("wqT", "wkT", "wvT", "woT"):
        io[nm] = nc.dram_tensor(nm, [D, D], bf16, kind="ExternalInput").ap()
    io["cst"] = nc.dram_tensor("cst", [128, 8], f32, kind="ExternalInput").ap()
    for nm in ("bv", "bo"):
        io[nm] = nc.dram_tensor(nm, [D], f32, kind="ExternalInput").ap()
    io["wdh"] = nc.dram_tensor("wdh", [KB, NI * SP], bf16,
                               kind="ExternalInput").ap()
    io["out"] = nc.dram_tensor("out", [B_LOC, S, N, D], f32,
                               kind="ExternalOutput").ap()

    with tile.TileContext(nc) as tc:
        with ExitStack() as ctx:
            emit_kernel(ctx, tc, io)
    nc.compile()
    return nc


def host_prep(Wq, bq, Wk, bk, Wv, bv, Wd, bd, Wo, bo):
    """Pre-transpose weights (bf16); fold the qk scale into Wq; build the
    kron weight for the bias L->S projection; pack bq/bk per-partition.
    bd cancels in the softmax (constant along the normalized axis)."""
    bf = ml_dtypes.bfloat16
    scale = (D // H) ** -0.5
    prep = {
        "wqT": np.ascontiguousarray((np.asarray(Wq) * scale).T).astype(bf),
        "wkT": np.ascontiguousarray(np.asarray(Wk).T).astype(bf),
        "wvT": np.ascontiguousarray(np.asarray(Wv).T).astype(bf),
        "woT": np.ascontiguousarray(np.asarray(Wo).T).astype(bf),
        "bv": np.asarray(bv, np.float32),
        "bo": np.asarray(bo, np.float32),
    }
    cst = np.zeros((128, 8), np.float32)
    for c in range(C):
        cst[:, c] = np.asarray(bq[c * 128:(c + 1) * 128]) * scale
        cst[:, 4 + c] = np.asarray(bk[c * 128:(c + 1) * 128])
    prep["cst"] = cst
    # wdh[(l*NI+ni), (ni*SP+s)] = Wd[s, l]; rows 96..127 zero pad (FWL)
    wdh = np.zeros((KB, NI * SP), np.float32)
    WdT = np.asarray(Wd, np.float32)  # [S, L]
    for l in range(L):
        for ni in range(NI):
            wdh[l * NI + ni, ni * SP:ni * SP + S] = WdT[:, l]
    prep["wdh"] = wdh.astype(bf)
    return prep


_NC_CACHE = None


def run(q, k, v, attn_bias, Wq, bq, Wk, bk, Wv, bv, Wd, bd, Wo, bo,
        trace=False, **trace_kwargs):
    global _NC_CACHE
    from concourse.bass_utils import run_bass_kernel_spmd

    if _NC_CACHE is None:
        _NC_CACHE = build_nc()
    nc = _NC_CACHE

    bf = ml_dtypes.bfloat16
    prep = host_prep(Wq, bq, Wk, bk, Wv, bv, Wd, bd, Wo, bo)
    # [B, D, N, S] -> [B, D, S, N] so on-chip token order is (s, n)
    q = np.ascontiguousarray(np.asarray(q, np.float32).transpose(0, 1, 3, 2)).astype(bf)
    k = np.ascontiguousarray(np.asarray(k, np.float32).transpose(0, 1, 3, 2)).astype(bf)
    v = np.ascontiguousarray(np.asarray(v, np.float32).transpose(0, 1, 3, 2)).astype(bf)
    # ab [B, L, n, m, H] -> kron layout [B, H, (L, ni), (nb, m)]
    ab = np.asarray(attn_bias, np.float32)
    B = ab.shape[0]
    abk = ab.transpose(0, 4, 1, 2, 3)                 # [B, H, L, n, m]
    abk = abk.reshape(B, H, L, NBK, NI, N)            # n -> (nb, ni)
    abk = abk.transpose(0, 1, 2, 4, 3, 5)             # [B, H, L, ni, nb, m]
    abk = np.ascontiguousarray(abk).reshape(B, H, K96, NBK * 128).astype(bf)
    abk = np.concatenate(
        [abk, np.zeros((B, H, KB - K96, NBK * 128), bf)], axis=2)

    in_maps = []
    for i in range(NCORES):
        sl = slice(i * B_LOC, (i + 1) * B_LOC)
        in_maps.append({
            "q": np.ascontiguousarray(q[sl]),
            "k": np.ascontiguousarray(k[sl]),
            "v": np.ascontiguousarray(v[sl]),
            "abk": np.ascontiguousarray(abk[sl]),
            **prep,
        })
    res = run_bass_kernel_spmd(nc, in_maps, list(range(NCORES)), trace=trace,
                               **trace_kwargs)
    out = np.concatenate([res.results[i]["out"] for i in range(NCORES)], axis=0)
    return out, res


def kernel(**inputs):
    return run(**inputs)[0]


# revision 44
# speedup vs baseline: 1.0193x; 1.0193x over previous
"""Trainium2 Bass kernel for nn_MultiHeadAttention_86079734546451.

Sharding: data-parallel over batch B=16 across 8 cores (2 batches/core).
All weights replicated. No collectives.

Per-core math (B_loc=2, D=512, N=128 nodes, S=14, L=12, H=8, dh=64):
  qh/kh = d-major [dout, (n,s)] bf16 projections (scale folded into Wq).
  bias  = einsum('lnmh,sl->mns', ab, Wd) via per-(h,nb) kron matmuls:
     lhsT = host-pretransposed ab block [(l,ni)=96, m=128] bf16,
     rhs  = kron Wd [96, (ni,s16)];  4 blocks -> one [128,512] psum tile,
     ACT-copied (strided, dropping the s-pad) into the score psum as the
     accumulation INIT; score matmuls then accumulate with start=False.
     (bd cancels in the softmax and is dropped.)
  softmax over the query axis == free-axis softmax in the [m,(s,n)] layout:
     one exp [128,1792] -> ebt bf16, 3D reduce -> Z, reciprocal,
     gpsimd broadcast-multiply onto vh -> vp; AV matmuls per (h,s);
     O-projection from the d-major yt tiles.
"""

import sys

sys.path.insert(0, "/opt/trn_rl_repo")

from contextlib import ExitStack

import numpy as np
import ml_dtypes

import concourse.bass as bass
import concourse.mybir as mybir
import concourse.tile as tile
from concourse import bacc
from concourse.bass import broadcast_tensor_aps

f32 = mybir.dt.float32
bf16 = mybir.dt.bfloat16
AF = mybir.ActivationFunctionType

# Problem constants
B_LOC = 2          # batches per core
D = 512
N = 128            # nodes
S = 14             # seq
SP = 14            # kron weight s-extent (no pad; 4x448B fits one bank)
L = 12
H = 8
DH = 64            # head dim
TOK = N * S        # 1792 tokens per batch, (n, s) order
C = 4              # 128-chunks of D
NCORES = 8
NI = 8             # n per kron block
NBK = N // NI      # 16 kron blocks
K96 = L * NI       # kron contraction size (real rows)
KB = 128           # padded kron K (zero rows 96..127) -> enables PE FWL


def emit_kernel(ctx: ExitStack, tc: "tile.TileContext", io: dict):
    nc = tc.nc

    q_d, k_d, v_d, ab_d = io["q"], io["k"], io["v"], io["abk"]
    out_d = io["out"]

    # ---------------- pools ----------------
    wpool = ctx.enter_context(tc.tile_pool(name="wpool", bufs=1))
    xin = ctx.enter_context(tc.tile_pool(name="xin", bufs=14))
    qkh = ctx.enter_context(tc.tile_pool(name="qkh", bufs=16))
    vhp = ctx.enter_context(tc.tile_pool(name="vhp", bufs=1))
    abp = ctx.enter_context(tc.tile_pool(name="abp", bufs=3))
    ebp = ctx.enter_context(tc.tile_pool(name="ebp", bufs=3))
    vpp = ctx.enter_context(tc.tile_pool(name="vpp", bufs=3))
    zrp = ctx.enter_context(tc.tile_pool(name="zrp", bufs=4))
    ytp = ctx.enter_context(tc.tile_pool(name="ytp", bufs=4))
    osbp = ctx.enter_context(tc.tile_pool(name="osbp", bufs=3))
    bhp = ctx.enter_context(tc.tile_pool(name="bhp", bufs=2))

    pp = ctx.enter_context(tc.tile_pool(name="pp", bufs=2, space="PSUM"))
    scp = ctx.enter_context(tc.tile_pool(name="scp", bufs=2, space="PSUM"))
    avp = ctx.enter_context(tc.tile_pool(name="avp", bufs=2, space="PSUM"))

    # ---------------- weights (once) ----------------
    wq, wk, wv, wo = [], [], [], []
    for nm, lst in (("wqT", wq), ("wkT", wk), ("wvT", wv), ("woT", wo)):
        for c in range(C):
            w_c = wpool.tile([128, D], bf16, name=f"{nm}{c}", tag=f"{nm}{c}")
            nc.scalar.dma_start(w_c[:], io[nm][c * 128:(c + 1) * 128, :])
            lst.append(w_c)

    wdh = wpool.tile([KB, NI * SP], bf16, name="wdh", tag="wdh")
    nc.scalar.dma_start(wdh[:], io["wdh"][:])

    # packed per-partition consts: cols 0-3 bq chunks, 4-7 bk chunks
    cst = wpool.tile([128, 8], f32, name="cst", tag="cst")
    nc.scalar.dma_start(cst[:], io["cst"][:])

    # broadcast bv/bo along partitions via K=1 ones-matmul
    ones = wpool.tile([1, 128], f32, name="ones", tag="ones")
    nc.vector.memset(ones[:], 1.0)
    bv_st = wpool.tile([1, D], f32, name="bv_st", tag="bv_st")
    nc.scalar.dma_start(bv_st[:], io["bv"].unsqueeze(0))
    bo_st = wpool.tile([1, D], f32, name="bo_st", tag="bo_st")
    nc.scalar.dma_start(bo_st[:], io["bo"].unsqueeze(0))
    bvb = wpool.tile([128, D], f32, name="bvb", tag="bvb")
    bob = wpool.tile([128, D], f32, name="bob", tag="bob")
    for src, dst in ((bv_st, bvb), (bo_st, bob)):
        pst = pp.tile([128, D], f32, tag="pp", name="p_bcast")
        nc.tensor.matmul(pst[:], lhsT=ones[:], rhs=src[:], start=True, stop=True)
        nc.vector.tensor_copy(dst[:], pst[:])

    def load_x(b, split=False):
        # x tiles are [128, (s, n)] -- host pre-transposed to [B, D, S, N]
        # split=True loads each chunk in column halves so the first proj
        # matmuls can start before the whole tile has landed (cold start)
        xs = {}
        for (src_d, nm) in ((q_d, "xq"), (k_d, "xk"), (v_d, "xv")):
            lst = []
            for ci in range(C):
                x_c = xin.tile([128, TOK], bf16, tag="xin", name=f"{nm}{ci}")
                src = src_d[b, ci * 128:(ci + 1) * 128].rearrange(
                    "p s n -> p (s n)")
                if split and nm != "xv":
                    nc.sync.dma_start(x_c[:, :448], src[:, :448])
                    nc.sync.dma_start(x_c[:, 448:896], src[:, 448:896])
                    nc.sync.dma_start(x_c[:, 896:], src[:, 896:])
                else:
                    nc.sync.dma_start(x_c[:], src)
                lst.append(x_c)
            xs[nm] = lst
        return xs

    def load_ab(b, h):
        ab_h = abp.tile([KB, NBK * 128], bf16, tag="abt", name=f"ab{h}")
        nc.sync.dma_start(ab_h[:], ab_d[b, h])
        return ab_h

    def emit_p1qk(xq, xk):
        # Q and K projections -> d-major [dout, (s,n)] bf16.
        # 4 concurrent psum accumulation chains (2 from pp + 2 from the
        # P3-idle avp pool) so the PE streams without RAW stalls
        qh, kh = [], []
        for pi, (xin_l, wts, dst_list) in enumerate(
                ((xq, wq, qh), (xk, wk, kh))):
            for co in range(C):
                h_c = qkh.tile([128, TOK], bf16, tag="qkh", name=f"h{co}")
                pss = [(pp if tb < 2 else avp).tile(
                    [128, 448], f32, tag="pp" if tb < 2 else "av",
                    name="ps_qk") for tb in range(4)]
                for ci in range(C):
                    for tb in range(4):
                        nc.tensor.matmul(
                            pss[tb][:],
                            lhsT=wts[ci][:, co * 128:(co + 1) * 128],
                            rhs=xin_l[ci][:, tb * 448:(tb + 1) * 448],
                            start=(ci == 0), stop=(ci == C - 1))
                for tb in range(4):
                    if pi == 0:
                        nc.scalar.activation(
                            h_c[:, tb * 448:(tb + 1) * 448], pss[tb][:],
                            AF.Identity, bias=cst[:, co:co + 1], scale=1.0)
                    else:
                        nc.vector.tensor_scalar_add(
                            h_c[:, tb * 448:(tb + 1) * 448], pss[tb][:],
                            cst[:, 4 + co:4 + co + 1])
                dst_list.append(h_c)
        return qh, kh

    # ---------------- per-batch body ----------------
    xs = load_x(0, split=True)
    qk_pre = None
    for b in range(B_LOC):
        xq, xk, xv = xs["xq"], xs["xk"], xs["xv"]

        # ---- P1: Q/K (emitted early during the previous batch's tail)
        qh, kh = qk_pre if qk_pre is not None else emit_p1qk(xq, xk)

        # ---- P1b: V projection -> token-major vh [n, (s,d)] bf16 (+bv)
        vh = vhp.tile([128, S * D], bf16, tag="vh", name="vh")
        for s0 in range(0, S, 4):
            sw = min(4, S - s0)
            pss = [(pp if sj < 2 else avp).tile(
                [128, D], f32, tag="pp" if sj < 2 else "av",
                name="ps_v") for sj in range(sw)]
            for ci in range(C):
                for sj in range(sw):
                    s = s0 + sj
                    nc.tensor.matmul(
                        pss[sj][:],
                        lhsT=xv[ci][:, s * 128:(s + 1) * 128],
                        rhs=wv[ci][:],
                        start=(ci == 0), stop=(ci == C - 1))
            for sj in range(sw):
                s = s0 + sj
                nc.vector.tensor_add(vh[:, s * D:(s + 1) * D], pss[sj][:],
                                     bvb[:])

        # first two ab loads ahead of the bulky next-batch x prefetch
        abL = {hh: load_ab(b, hh) for hh in range(2)}
        # prefetch next batch's inputs (sync queue, before this batch's outs)
        if b + 1 < B_LOC:
            xs = load_x(b + 1)

        # ---- P3: attention per head
        yt = [ytp.tile([128, TOK], bf16, tag="ytp", name=f"yt{c}")
              for c in range(C)]
        ebts, vps = {}, {}

        def av_block(pair):
            # heads of a pair write distinct out-partition halves (col_grp
            # 0 vs 64) -- interleave them pairwise so the PE co-runs them
            cc = pair
            for g in range(4):
                s0 = g * 4
                sw = min(4, S - s0)
                av = avp.tile([128, 512], f32, tag="av", name="av")
                for si in range(sw):
                    s = s0 + si
                    for hh in (2 * cc, 2 * cc + 1):
                        hbb = (hh % 2) * DH
                        nc.tensor.matmul(
                            av[hbb:hbb + DH, si * 128:(si + 1) * 128],
                            lhsT=vps[hh][:, s * DH:(s + 1) * DH],
                            rhs=ebts[hh][:, s * 128:(s + 1) * 128],
                            start=True, stop=True, skip_group_check=True)
                nc.scalar.copy(
                    yt[cc][:, s0 * 128:(s0 + sw) * 128],
                    av[:, :sw * 128])

        for h in range(H):
            c = h // 2
            hb = (h % 2) * DH
            if h + 2 < H:
                abL[h + 2] = load_ab(b, h + 2)
            # bias kron matmuls -> [128,448] psum tiles -> one sbuf bf16
            # tile [m, (n,s)] per head (engine-write -> PE-accumulate
            # proved racy on HW, so DVE adds the bias after the score mms)
            bias_sb = bhp.tile([128, N * SP], bf16, tag="bh", name="bias_sb")
            for g in range(4):
                psb = pp.tile([128, 4 * NI * SP], f32, tag="pp", name="ps_b")
                for j in range(4):
                    nb = g * 4 + j
                    nc.tensor.matmul(
                        psb[:, j * NI * SP:(j + 1) * NI * SP],
                        lhsT=abL[h][:, nb * 128:(nb + 1) * 128],
                        rhs=wdh[:],
                        start=True, stop=True, skip_group_check=True)
                nc.scalar.copy(
                    bias_sb[:, g * 4 * NI * SP:(g + 1) * 4 * NI * SP],
                    psb[:])
            # AV of the previous pair slots in here: its vp/ebt chain has
            # had a full head iteration to finish, so PE never stalls on it
            if h % 2 == 0 and h >= 2:
                av_block(h // 2 - 1)
            # scores + softmax, split into s-halves A (s 0..6) and B
            # (s 7..13) so the next head's A matmuls only wait on exp(A)
            ebt = ebp.tile([128, TOK], bf16, tag="eb", name="ebt")
            zt = zrp.tile([128, S], f32, tag="z", name="zt")
            rt = zrp.tile([128, S], f32, tag="r", name="rt")
            vp = vpp.tile([128, S * DH], bf16, tag="vp", name="vp")
            HS = S // 2  # 7
            schs = [scp.tile([128, HS * 128], f32, tag="sc", name="sc")
                    for _ in range(2)]
            for half in range(2):
                sch = schs[half]
                sb = half * HS
                for si in range(HS):
                    s = sb + si
                    nc.tensor.matmul(
                        sch[:, si * 128:(si + 1) * 128],
                        lhsT=kh[c][hb:hb + DH, s * 128:(s + 1) * 128],
                        rhs=qh[c][hb:hb + DH, s * 128:(s + 1) * 128],
                        start=True, stop=True, skip_group_check=True)
            for half in range(2):
                sch = schs[half]
                sb = half * HS
                sc_ns = sch.rearrange("p (s n) -> p n s", s=HS)
                nc.vector.tensor_add(
                    sc_ns[:],
                    sc_ns[:],
                    bias_sb.rearrange("p (n s) -> p n s", s=SP)[
                        :, :, sb:sb + HS])
                nc.scalar.activation(ebt[:, sb * 128:(sb + HS) * 128],
                                     sch[:], AF.Exp)
                nc.vector.reduce_sum(
                    zt[:, sb:sb + HS].unsqueeze(2),
                    ebt.rearrange("p (s n) -> p s n", s=S)[:, sb:sb + HS],
                    axis=mybir.AxisListType.X)
                nc.vector.reciprocal(rt[:, sb:sb + HS], zt[:, sb:sb + HS])
                vsrc = vh.rearrange("p (s d) -> p s d", s=S)[
                    :, sb:sb + HS, h * DH:(h + 1) * DH]
                a2, b2 = broadcast_tensor_aps(
                    vsrc, rt[:, sb:sb + HS].unsqueeze(2))
                nc.gpsimd.tensor_mul(
                    vp.rearrange("p (s d) -> p s d", s=S)[:, sb:sb + HS],
                    a2, b2)
            ebts[h], vps[h] = ebt, vp
        # fill the last pair's chain latency with next batch's QK proj
        qk_pre = emit_p1qk(xs["xq"], xs["xk"]) if b + 1 < B_LOC else None
        av_block(3)

        # ---- P4: output projection (4 concurrent psum chains)
        for s0 in range(0, S, 4):
            sw = min(4, S - s0)
            pss = [(pp if sj < 2 else avp).tile(
                [128, D], f32, tag="pp" if sj < 2 else "av",
                name="ps_o") for sj in range(sw)]
            for ci in range(C):
                for sj in range(sw):
                    s = s0 + sj
                    nc.tensor.matmul(
                        pss[sj][:],
                        lhsT=yt[ci][:, s * 128:(s + 1) * 128],
                        rhs=wo[ci][:],
                        start=(ci == 0), stop=(ci == C - 1))
            for sj in range(sw):
                s = s0 + sj
                osb = osbp.tile([128, D], f32, tag="osb", name="osb")
                nc.vector.tensor_add(osb[:], pss[sj][:], bob[:])
                nc.sync.dma_start(out_d[b, s], osb[:])


def build_nc():
    nc = bacc.Bacc("TRN2", target_bir_lowering=False, debug=False,
                   num_devices=NCORES)
    io = {}
    io["q"] = nc.dram_tensor("q", [B_LOC, D, S, N], bf16, kind="ExternalInput").ap()
    io["k"] = nc.dram_tensor("k", [B_LOC, D, S, N], bf16, kind="ExternalInput").ap()
    io["v"] = nc.dram_tensor("v", [B_LOC, D, S, N], bf16, kind="ExternalInput").ap()
    io["abk"] = nc.dram_tensor("abk", [B_LOC, H, KB, NBK * 128], bf16,
                               kind="ExternalInput").ap()
    for nm in ("wqT", "wkT", "wvT", "woT"):
        io[nm] = nc.dram_tensor(nm, [D, D], bf16, kind="ExternalInput").ap()
    io["cst"] = nc.dram_tensor("cst", [128, 8], f32, kind="ExternalInput").ap()
    for nm in ("bv", "bo"):
        io[nm] = nc.dram_tensor(nm, [D], f32, kind="ExternalInput").ap()
    io["wdh"] = nc.dram_tensor("wdh", [KB, NI * SP], bf16,
                               kind="ExternalInput").ap()
    io["out"] = nc.dram_tensor("out", [B_LOC, S, N, D], f32,
                               kind="ExternalOutput").ap()

    with tile.TileContext(nc) as tc:
        with ExitStack() as ctx:
            emit_kernel(ctx, tc, io)
    nc.compile()
    return nc


def host_prep(Wq, bq, Wk, bk, Wv, bv, Wd, bd, Wo, bo):
    """Pre-transpose weights (bf16); fold the qk scale into Wq; build the
    kron weight for the bias L->S projection; pack bq/bk per-partition.
    bd cancels in the softmax (constant along the normalized axis)."""
    bf = ml_dtypes.bfloat16
    scale = (D // H) ** -0.5
    prep = {
        "wqT": np.ascontiguousarray((np.asarray(Wq) * scale).T).astype(bf),
        "wkT": np.ascontiguousarray(np.asarray(Wk).T).astype(bf),
        "wvT": np.ascontiguousarray(np.asarray(Wv).T).astype(bf),
        "woT": np.ascontiguousarray(np.asarray(Wo).T).astype(bf),
        "bv": np.asarray(bv, np.float32),
        "bo": np.asarray(bo, np.float32),
    }
    cst = np.zeros((128, 8), np.float32)
    for c in range(C):
        cst[:, c] = np.asarray(bq[c * 128:(c + 1) * 128]) * scale
        cst[:, 4 + c] = np.asarray(bk[c * 128:(c + 1) * 128])
    prep["cst"] = cst
    # wdh[(l*NI+ni), (ni*SP+s)] = Wd[s, l]; rows 96..127 zero pad (FWL)
    wdh = np.zeros((KB, NI * SP), np.float32)
    WdT = np.asarray(Wd, np.float32)  # [S, L]
    for l in range(L):
        for ni in range(NI):
            wdh[l * NI + ni, ni * SP:ni * SP + S] = WdT[:, l]
    prep["wdh"] = wdh.astype(bf)
    return prep


_NC_CACHE = None


def run(q, k, v, attn_bias, Wq, bq, Wk, bk, Wv, bv, Wd, bd, Wo, bo,
        trace=False, **trace_kwargs):
    global _NC_CACHE
    from concourse.bass_utils import run_bass_kernel_spmd

    if _NC_CACHE is None:
        _NC_CACHE = build_nc()
    nc = _NC_CACHE

    bf = ml_dtypes.bfloat16
    prep = host_prep(Wq, bq, Wk, bk, Wv, bv, Wd, bd, Wo, bo)
    # [B, D, N, S] -> [B, D, S, N] so on-chip token order is (s, n)
    q = np.ascontiguousarray(np.asarray(q, np.float32).transpose(0, 1, 3, 2)).astype(bf)
    k = np.ascontiguousarray(np.asarray(k, np.float32).transpose(0, 1, 3, 2)).astype(bf)
    v = np.ascontiguousarray(np.asarray(v, np.float32).transpose(0, 1, 3, 2)).astype(bf)
    # ab [B, L, n, m, H] -> kron layout [B, H, (L, ni), (nb, m)]
    ab = np.asarray(attn_bias, np.float32)
    B = ab.shape[0]
    abk = ab.transpose(0, 4, 1, 2, 3)                 # [B, H, L, n, m]
    abk = abk.reshape(B, H, L, NBK, NI, N)            # n -> (nb, ni)
    abk = abk.transpose(0, 1, 2, 4, 3, 5)             # [B, H, L, ni, nb, m]
    abk = np.ascontiguousarray(abk).reshape(B, H, K96, NBK * 128).astype(bf)
    abk = np.concatenate(
        [abk, np.zeros((B, H, KB - K96, NBK * 128), bf)], axis=2)

    in_maps = []
    for i in range(NCORES):
        sl = slice(i * B_LOC, (i + 1) * B_LOC)
        in_maps.append({
            "q": np.ascontiguousarray(q[sl]),
            "k": np.ascontiguousarray(k[sl]),
            "v": np.ascontiguousarray(v[sl]),
            "abk": np.ascontiguousarray(abk[sl]),
            **prep,
        })
    res = run_bass_kernel_spmd(nc, in_maps, list(range(NCORES)), trace=trace,
                               **trace_kwargs)
    out = np.concatenate([res.results[i]["out"] for i in range(NCORES)], axis=0)
    return out, res


def kernel(**inputs):
    return run(**inputs)[0]


# revision 45
# speedup vs baseline: 1.0494x; 1.0296x over previous
"""Trainium2 Bass kernel for nn_MultiHeadAttention_86079734546451.

Sharding: data-parallel over batch B=16 across 8 cores (2 batches/core).
All weights replicated. No collectives.

Per-core structure (B_loc=2, D=512, N=128 nodes, S=14, L=12, H=8, dh=64),
everything bf16 with f32 psum; token order is (s, n) via a host transpose:
  P1  Q/K projections -> d-major [dout, (s,n)] tiles; V projection ->
      token-major vh [n, (s,d)] (x as lhsT). All projections run 4
      concurrent psum accumulation chains (2 pp + 2 avp banks) so the PE
      streams without RAW stalls; the next batch's Q/K projection is
      emitted during the previous batch's attention tail.
  P3  per head: bias = einsum('lnmh,sl->mns', ab, Wd) via kron matmuls
      (lhsT = host-pretransposed ab block [(l,ni) pad 128, m], rhs = kron
      Wd [128, (ni,s)]) -> one bf16 bias tile [m,(n,s)]; scores
      kh.qh -> two [128, 896] psum half-tiles; DVE adds the bias (an
      ACT-init + matmul-accumulate variant was faster on paper but
      proved racy on HW); ACT exp -> ebt bf16; DVE 3D reduce -> Z;
      reciprocal; gpsimd broadcast-multiply -> vp = vh/Z. AV matmuls are
      emitted one head-pair late so the PE never stalls on the softmax
      chain, with the pair's heads interleaved (distinct out-partition
      halves). bd cancels in the softmax and is dropped.
  P4  O-projection from the d-major yt tiles, bias added in the psum
      drain, stores via the sync queue.
"""

import sys

sys.path.insert(0, "/opt/trn_rl_repo")

from contextlib import ExitStack

import numpy as np
import ml_dtypes

import concourse.bass as bass
import concourse.mybir as mybir
import concourse.tile as tile
from concourse import bacc
from concourse.bass import broadcast_tensor_aps

f32 = mybir.dt.float32
bf16 = mybir.dt.bfloat16
AF = mybir.ActivationFunctionType

# Problem constants
B_LOC = 2          # batches per core
D = 512
N = 128            # nodes
S = 14             # seq
SP = 14            # kron weight s-extent (no pad; 4x448B fits one bank)
L = 12
H = 8
DH = 64            # head dim
TOK = N * S        # 1792 tokens per batch, (n, s) order
C = 4              # 128-chunks of D
NCORES = 8
NI = 8             # n per kron block
NBK = N // NI      # 16 kron blocks
K96 = L * NI       # kron contraction size (real rows)
KB = 128           # padded kron K (zero rows 96..127) -> enables PE FWL


def emit_kernel(ctx: ExitStack, tc: "tile.TileContext", io: dict):
    nc = tc.nc

    q_d, k_d, v_d, ab_d = io["q"], io["k"], io["v"], io["abk"]
    out_d = io["out"]

    # ---------------- pools ----------------
    wpool = ctx.enter_context(tc.tile_pool(name="wpool", bufs=1))
    xin = ctx.enter_context(tc.tile_pool(name="xin", bufs=14))
    qkh = ctx.enter_context(tc.tile_pool(name="qkh", bufs=16))
    vhp = ctx.enter_context(tc.tile_pool(name="vhp", bufs=1))
    abp = ctx.enter_context(tc.tile_pool(name="abp", bufs=3))
    ebp = ctx.enter_context(tc.tile_pool(name="ebp", bufs=3))
    vpp = ctx.enter_context(tc.tile_pool(name="vpp", bufs=3))
    zrp = ctx.enter_context(tc.tile_pool(name="zrp", bufs=4))
    ytp = ctx.enter_context(tc.tile_pool(name="ytp", bufs=4))
    osbp = ctx.enter_context(tc.tile_pool(name="osbp", bufs=3))
    bhp = ctx.enter_context(tc.tile_pool(name="bhp", bufs=2))

    pp = ctx.enter_context(tc.tile_pool(name="pp", bufs=2, space="PSUM"))
    scp = ctx.enter_context(tc.tile_pool(name="scp", bufs=2, space="PSUM"))
    avp = ctx.enter_context(tc.tile_pool(name="avp", bufs=2, space="PSUM"))

    # ---------------- weights (once) ----------------
    wq, wk, wv, wo = [], [], [], []
    for nm, lst in (("wqT", wq), ("wkT", wk), ("wvT", wv), ("woT", wo)):
        for c in range(C):
            w_c = wpool.tile([128, D], bf16, name=f"{nm}{c}", tag=f"{nm}{c}")
            nc.scalar.dma_start(w_c[:], io[nm][c * 128:(c + 1) * 128, :])
            lst.append(w_c)

    wdh = wpool.tile([KB, NI * SP], bf16, name="wdh", tag="wdh")
    nc.scalar.dma_start(wdh[:], io["wdh"][:])

    # packed per-partition consts: cols 0-3 bq chunks, 4-7 bk chunks
    cst = wpool.tile([128, 8], f32, name="cst", tag="cst")
    nc.scalar.dma_start(cst[:], io["cst"][:])

    # broadcast bv/bo along partitions via K=1 ones-matmul
    ones = wpool.tile([1, 128], f32, name="ones", tag="ones")
    nc.vector.memset(ones[:], 1.0)
    bv_st = wpool.tile([1, D], f32, name="bv_st", tag="bv_st")
    nc.scalar.dma_start(bv_st[:], io["bv"].unsqueeze(0))
    bo_st = wpool.tile([1, D], f32, name="bo_st", tag="bo_st")
    nc.scalar.dma_start(bo_st[:], io["bo"].unsqueeze(0))
    bvb = wpool.tile([128, D], f32, name="bvb", tag="bvb")
    bob = wpool.tile([128, D], f32, name="bob", tag="bob")
    for src, dst in ((bv_st, bvb), (bo_st, bob)):
        pst = pp.tile([128, D], f32, tag="pp", name="p_bcast")
        nc.tensor.matmul(pst[:], lhsT=ones[:], rhs=src[:], start=True, stop=True)
        nc.vector.tensor_copy(dst[:], pst[:])

    def load_x(b, split=False):
        # x tiles are [128, (s, n)] -- host pre-transposed to [B, D, S, N]
        # split=True loads each chunk in column halves so the first proj
        # matmuls can start before the whole tile has landed (cold start)
        xs = {}
        for (src_d, nm) in ((q_d, "xq"), (k_d, "xk"), (v_d, "xv")):
            lst = []
            for ci in range(C):
                x_c = xin.tile([128, TOK], bf16, tag="xin", name=f"{nm}{ci}")
                src = src_d[b, ci * 128:(ci + 1) * 128].rearrange(
                    "p s n -> p (s n)")
                if split and nm != "xv":
                    nc.sync.dma_start(x_c[:, :448], src[:, :448])
                    nc.sync.dma_start(x_c[:, 448:896], src[:, 448:896])
                    nc.sync.dma_start(x_c[:, 896:], src[:, 896:])
                else:
                    nc.sync.dma_start(x_c[:], src)
                lst.append(x_c)
            xs[nm] = lst
        return xs

    def load_ab(b, h):
        ab_h = abp.tile([KB, NBK * 128], bf16, tag="abt", name=f"ab{h}")
        nc.sync.dma_start(ab_h[:], ab_d[b, h])
        return ab_h

    def emit_p1qk(xq, xk):
        # Q and K projections -> d-major [dout, (s,n)] bf16.
        # 4 concurrent psum accumulation chains (2 from pp + 2 from the
        # P3-idle avp pool) so the PE streams without RAW stalls
        qh, kh = [], []
        for pi, (xin_l, wts, dst_list) in enumerate(
                ((xq, wq, qh), (xk, wk, kh))):
            for co in range(C):
                h_c = qkh.tile([128, TOK], bf16, tag="qkh", name=f"h{co}")
                pss = [(pp if tb < 2 else avp).tile(
                    [128, 448], f32, tag="pp" if tb < 2 else "av",
                    name="ps_qk") for tb in range(4)]
                for ci in range(C):
                    for tb in range(4):
                        nc.tensor.matmul(
                            pss[tb][:],
                            lhsT=wts[ci][:, co * 128:(co + 1) * 128],
                            rhs=xin_l[ci][:, tb * 448:(tb + 1) * 448],
                            start=(ci == 0), stop=(ci == C - 1))
                for tb in range(4):
                    if pi == 0:
                        nc.scalar.activation(
                            h_c[:, tb * 448:(tb + 1) * 448], pss[tb][:],
                            AF.Identity, bias=cst[:, co:co + 1], scale=1.0)
                    else:
                        nc.vector.tensor_scalar_add(
                            h_c[:, tb * 448:(tb + 1) * 448], pss[tb][:],
                            cst[:, 4 + co:4 + co + 1])
                dst_list.append(h_c)
        return qh, kh

    # ---------------- per-batch body ----------------
    xs = load_x(0, split=True)
    qk_pre = None
    for b in range(B_LOC):
        xq, xk, xv = xs["xq"], xs["xk"], xs["xv"]

        # ---- P1: Q/K (emitted early during the previous batch's tail)
        qh, kh = qk_pre if qk_pre is not None else emit_p1qk(xq, xk)

        # ---- P1b: V projection -> token-major vh [n, (s,d)] bf16 (+bv)
        vh = vhp.tile([128, S * D], bf16, tag="vh", name="vh")
        for s0 in range(0, S, 4):
            sw = min(4, S - s0)
            pss = [(pp if sj < 2 else avp).tile(
                [128, D], f32, tag="pp" if sj < 2 else "av",
                name="ps_v") for sj in range(sw)]
            for ci in range(C):
                for sj in range(sw):
                    s = s0 + sj
                    nc.tensor.matmul(
                        pss[sj][:],
                        lhsT=xv[ci][:, s * 128:(s + 1) * 128],
                        rhs=wv[ci][:],
                        start=(ci == 0), stop=(ci == C - 1))
            for sj in range(sw):
                s = s0 + sj
                nc.vector.tensor_add(vh[:, s * D:(s + 1) * D], pss[sj][:],
                                     bvb[:])

        # first two ab loads ahead of the bulky next-batch x prefetch
        abL = {hh: load_ab(b, hh) for hh in range(2)}
        # prefetch next batch's inputs (sync queue, before this batch's outs)
        if b + 1 < B_LOC:
            xs = load_x(b + 1)

        # ---- P3: attention per head
        yt = [ytp.tile([128, TOK], bf16, tag="ytp", name=f"yt{c}")
              for c in range(C)]
        ebts, vps = {}, {}

        def av_block(pair):
            # heads of a pair write distinct out-partition halves (col_grp
            # 0 vs 64) -- interleave them pairwise so the PE co-runs them
            cc = pair
            for g in range(4):
                s0 = g * 4
                sw = min(4, S - s0)
                av = avp.tile([128, 512], f32, tag="av", name="av")
                for si in range(sw):
                    s = s0 + si
                    for hh in (2 * cc, 2 * cc + 1):
                        hbb = (hh % 2) * DH
                        nc.tensor.matmul(
                            av[hbb:hbb + DH, si * 128:(si + 1) * 128],
                            lhsT=vps[hh][:, s * DH:(s + 1) * DH],
                            rhs=ebts[hh][:, s * 128:(s + 1) * 128],
                            start=True, stop=True, skip_group_check=True)
                nc.scalar.copy(
                    yt[cc][:, s0 * 128:(s0 + sw) * 128],
                    av[:, :sw * 128])

        for h in range(H):
            c = h // 2
            hb = (h % 2) * DH
            if h + 2 < H:
                abL[h + 2] = load_ab(b, h + 2)
            # bias kron matmuls -> [128,448] psum tiles -> one sbuf bf16
            # tile [m, (n,s)] per head (engine-write -> PE-accumulate
            # proved racy on HW, so DVE adds the bias after the score mms)
            bias_sb = bhp.tile([128, N * SP], bf16, tag="bh", name="bias_sb")
            for g in range(4):
                psb = pp.tile([128, 4 * NI * SP], f32, tag="pp", name="ps_b")
                for j in range(4):
                    nb = g * 4 + j
                    nc.tensor.matmul(
                        psb[:, j * NI * SP:(j + 1) * NI * SP],
                        lhsT=abL[h][:, nb * 128:(nb + 1) * 128],
                        rhs=wdh[:],
                        start=True, stop=True, skip_group_check=True)
                nc.scalar.copy(
                    bias_sb[:, g * 4 * NI * SP:(g + 1) * 4 * NI * SP],
                    psb[:])
            # AV of the previous pair slots in here: its vp/ebt chain has
            # had a full head iteration to finish, so PE never stalls on it
            if h % 2 == 0 and h >= 2:
                av_block(h // 2 - 1)
            # scores + softmax, split into s-halves A (s 0..6) and B
            # (s 7..13) so the next head's A matmuls only wait on exp(A)
            ebt = ebp.tile([128, TOK], bf16, tag="eb", name="ebt")
            zt = zrp.tile([128, S], f32, tag="z", name="zt")
            rt = zrp.tile([128, S], f32, tag="r", name="rt")
            vp = vpp.tile([128, S * DH], bf16, tag="vp", name="vp")
            HS = S // 2  # 7
            schs = [scp.tile([128, HS * 128], f32, tag="sc", name="sc")
                    for _ in range(2)]
            for half in range(2):
                sch = schs[half]
                sb = half * HS
                for si in range(HS):
                    s = sb + si
                    nc.tensor.matmul(
                        sch[:, si * 128:(si + 1) * 128],
                        lhsT=kh[c][hb:hb + DH, s * 128:(s + 1) * 128],
                        rhs=qh[c][hb:hb + DH, s * 128:(s + 1) * 128],
                        start=True, stop=True, skip_group_check=True)
            for half in range(2):
                sch = schs[half]
                sb = half * HS
                sc_ns = sch.rearrange("p (s n) -> p n s", s=HS)
                nc.vector.tensor_add(
                    sc_ns[:],
                    sc_ns[:],
                    bias_sb.rearrange("p (n s) -> p n s", s=SP)[
                        :, :, sb:sb + HS])
                nc.scalar.activation(ebt[:, sb * 128:(sb + HS) * 128],
                                     sch[:], AF.Exp)
                nc.vector.reduce_sum(
                    zt[:, sb:sb + HS].unsqueeze(2),
                    ebt.rearrange("p (s n) -> p s n", s=S)[:, sb:sb + HS],
                    axis=mybir.AxisListType.X)
                nc.vector.reciprocal(rt[:, sb:sb + HS], zt[:, sb:sb + HS])
                vsrc = vh.rearrange("p (s d) -> p s d", s=S)[
                    :, sb:sb + HS, h * DH:(h + 1) * DH]
                a2, b2 = broadcast_tensor_aps(
                    vsrc, rt[:, sb:sb + HS].unsqueeze(2))
                nc.gpsimd.tensor_mul(
                    vp.rearrange("p (s d) -> p s d", s=S)[:, sb:sb + HS],
                    a2, b2)
            ebts[h], vps[h] = ebt, vp
        # fill the last pair's chain latency with next batch's QK proj
        qk_pre = emit_p1qk(xs["xq"], xs["xk"]) if b + 1 < B_LOC else None
        av_block(3)

        # ---- P4: output projection (4 concurrent psum chains)
        for s0 in range(0, S, 4):
            sw = min(4, S - s0)
            pss = [(pp if sj < 2 else avp).tile(
                [128, D], f32, tag="pp" if sj < 2 else "av",
                name="ps_o") for sj in range(sw)]
            for ci in range(C):
                for sj in range(sw):
                    s = s0 + sj
                    nc.tensor.matmul(
                        pss[sj][:],
                        lhsT=yt[ci][:, s * 128:(s + 1) * 128],
                        rhs=wo[ci][:],
                        start=(ci == 0), stop=(ci == C - 1))
            for sj in range(sw):
                s = s0 + sj
                osb = osbp.tile([128, D], f32, tag="osb", name="osb")
                nc.vector.tensor_add(osb[:], pss[sj][:], bob[:])
                nc.sync.dma_start(out_d[b, s], osb[:])


def build_nc():
    nc = bacc.Bacc("TRN2", target_bir_lowering=False, debug=False,
                   num_devices=NCORES)
    io = {}
    io["q"] = nc.dram_tensor("q", [B_LOC, D, S, N], bf16, kind="ExternalInput").ap()
    io["k"] = nc.dram_tensor("k", [B_LOC, D, S, N], bf16, kind="ExternalInput").ap()
    io["v"] = nc.dram_tensor("v", [B_LOC, D, S, N], bf16, kind="ExternalInput").ap()
    io["abk"] = nc.dram_tensor("abk", [B_LOC, H, KB, NBK * 128], bf16,
                               kind="ExternalInput").ap()
    for nm in ("wqT", "wkT", "wvT", "woT"):
        io[nm] = nc.dram_tensor(nm, [D, D], bf16, kind="ExternalInput").ap()
    io["cst"] = nc.dram_tensor("cst", [128, 8], f32, kind="ExternalInput").ap()
    for nm in ("bv", "bo"):
        io[nm] = nc.dram_tensor(nm, [D], f32, kind="ExternalInput").ap()
    io["wdh"] = nc.dram_tensor("wdh", [KB, NI * SP], bf16,
                               kind="ExternalInput").ap()
    io["out"] = nc.dram_tensor("out", [B_LOC, S, N, D], f32,
                               kind="ExternalOutput").ap()

    with tile.TileContext(nc) as tc:
        with ExitStack() as ctx:
            emit_kernel(ctx, tc, io)
    nc.compile()
    return nc


def host_prep(Wq, bq, Wk, bk, Wv, bv, Wd, bd, Wo, bo):
    """Pre-transpose weights (bf16); fold the qk scale into Wq; build the
    kron weight for the bias L->S projection; pack bq/bk per-partition.
    bd cancels in the softmax (constant along the normalized axis)."""
    bf = ml_dtypes.bfloat16
    scale = (D // H) ** -0.5
    prep = {
        "wqT": np.ascontiguousarray((np.asarray(Wq) * scale).T).astype(bf),
        "wkT": np.ascontiguousarray(np.asarray(Wk).T).astype(bf),
        "wvT": np.ascontiguousarray(np.asarray(Wv).T).astype(bf),
        "woT": np.ascontiguousarray(np.asarray(Wo).T).astype(bf),
        "bv": np.asarray(bv, np.float32),
        "bo": np.asarray(bo, np.float32),
    }
    cst = np.zeros((128, 8), np.float32)
    for c in range(C):
        cst[:, c] = np.asarray(bq[c * 128:(c + 1) * 128]) * scale
        cst[:, 4 + c] = np.asarray(bk[c * 128:(c + 1) * 128])
    prep["cst"] = cst
    # wdh[(l*NI+ni), (ni*SP+s)] = Wd[s, l]; rows 96..127 zero pad (FWL)
    wdh = np.zeros((KB, NI * SP), np.float32)
    WdT = np.asarray(Wd, np.float32)  # [S, L]
    for l in range(L):
        for ni in range(NI):
            wdh[l * NI + ni, ni * SP:ni * SP + S] = WdT[:, l]
    prep["wdh"] = wdh.astype(bf)
    return prep


_NC_CACHE = None


def run(q, k, v, attn_bias, Wq, bq, Wk, bk, Wv, bv, Wd, bd, Wo, bo,
        trace=False, **trace_kwargs):
    global _NC_CACHE
    from concourse.bass_utils import run_bass_kernel_spmd

    if _NC_CACHE is None:
        _NC_CACHE = build_nc()
    nc = _NC_CACHE

    bf = ml_dtypes.bfloat16
    prep = host_prep(Wq, bq, Wk, bk, Wv, bv, Wd, bd, Wo, bo)
    # [B, D, N, S] -> [B, D, S, N] so on-chip token order is (s, n)
    q = np.ascontiguousarray(np.asarray(q, np.float32).transpose(0, 1, 3, 2)).astype(bf)
    k = np.ascontiguousarray(np.asarray(k, np.float32).transpose(0, 1, 3, 2)).astype(bf)
    v = np.ascontiguousarray(np.asarray(v, np.float32).transpose(0, 1, 3, 2)).astype(bf)
    # ab [B, L, n, m, H] -> kron layout [B, H, (L, ni), (nb, m)]
    ab = np.asarray(attn_bias, np.float32)
    B = ab.shape[0]
    abk = ab.transpose(0, 4, 1, 2, 3)                 # [B, H, L, n, m]
    abk = abk.reshape(B, H, L, NBK, NI, N)            # n -> (nb, ni)
    abk = abk.transpose(0, 1, 2, 4, 3, 5)             # [B, H, L, ni, nb, m]
    abk = np.ascontiguousarray(abk).reshape(B, H, K96, NBK * 128).astype(bf)
    abk = np.concatenate(
        [abk, np.zeros((B, H, KB - K96, NBK * 128), bf)], axis=2)

    in_maps = []
    for i in range(NCORES):
        sl = slice(i * B_LOC, (i + 1) * B_LOC)
        in_maps.append({
            "q": np.ascontiguousarray(q[sl]),
            "k": np.ascontiguousarray(k[sl]),
            "v": np.ascontiguousarray(v[sl]),
            "abk": np.ascontiguousarray(abk[sl]),
            **prep,
        })
    res = run_bass_kernel_spmd(nc, in_maps, list(range(NCORES)), trace=trace,
                               **trace_kwargs)
    out = np.concatenate([res.results[i]["out"] for i in range(NCORES)], axis=0)
    return out, res


def kernel(**inputs):
    return run(**inputs)[0]
